# revision 14
# baseline (speedup 1.0000x reference)
"""Trainium2 Bass kernel for the DifferentiableQuantumCircuit problem.

Math: output = |U x / ||x|| |^2 with U = kron of 12 single-qubit U3 gates
applied twice (2 layers). Gates on different qubits commute, so the two
layers fuse into ONE kron-product unitary with per-qubit gates
G_q = U3_layer2(q) @ U3_layer1(q).

State index split: i = q5 * 128 + l7, with q5 = qubits 0-4 (5 MSBs) and
l7 = qubits 5-11 (7 LSBs, contiguous in memory -> 512B DMA bursts).
U_total = M5a (x) M7b with M5a = kron(G_0..G_4) [32x32] acting on q5 and
M7b = kron(G_5..G_11) [128x128] acting on l7.

Per-core dataflow (512 samples/core, 4 chunks of 128 samples b=(bh,b2),
bh in [0,32), b2 in [0,4)); per chunk, 8 groups of 4 bh:
  stage 1 (PE, f32r): stationary = X c-tile (fixed bh), moving =
    [Re(G5^T)|Im(G5^T)] with G5 = I4 (x) M5a -> psum[l7, (re/im,(b2,q5))]
    (applies the 5-qubit gate group AND transposes l7 onto partitions)
  evac (V/S): psum f32 -> SBUF bf16 S1 tiles
  stage 2 (PE, bf16): stationary = S1 re/im slices, moving =
    [Re(M7b^T)|Im(M7b^T)] / [-Im|Re] accumulating -> psum[(b2,q5'), (re/im, l7')]
  squares (S): psum f32 -> T bf16; pair add (V/G) -> Pf bf16
  norm (off critical path): x^2 (S) -> per-bh l7-reduce (V) ->
    block-diag-ones matmul (PE) -> reciprocal (V) = 1/||x||^2
  final scale (G): Pf * invnorm2 broadcast -> PfS f32 -> DMA store

SOFTWARE PIPELINE: engine queues are FIFO in emission order, so the
main loop k interleaves stage-2 of chunk k with stage-1 of chunk k+1.
Every cross-engine dependency then has ~a full chunk of slack instead
of serializing a per-group latency chain. Norm chain for chunk k+1 is
also emitted inside loop k, placed where its inputs have landed.
"""

from contextlib import ExitStack

import numpy as np
import ml_dtypes

import concourse.tile as tile
from concourse import bacc, mybir
from concourse.alu_op_type import AluOpType
from concourse.bass_utils import run_bass_kernel_spmd

F32 = mybir.dt.float32
F32R = mybir.dt.float32r
BF16 = mybir.dt.bfloat16

NUM_QUBITS = 12
D = 4096
B = 4096
N_CORES = 8
B_CORE = B // N_CORES  # 512
CHUNK = 128
N_CHUNKS = B_CORE // CHUNK  # 4
NG = 8  # groups per chunk (4 bh each)

EVAC_V = (0, 1, 2, 4, 5, 6)  # stage-1 evacuation on VectorE; rest ScalarE
ADD_V = (0, 1, 2)  # pair-adds on VectorE (early groups fill V's loop start)


def _u3(theta, phi, lam):
    c = np.cos(theta / 2.0)
    s = np.sin(theta / 2.0)
    return np.array(
        [
            [c, -np.exp(1j * lam) * s],
            [np.exp(1j * phi) * s, np.exp(1j * (phi + lam)) * c],
        ],
        dtype=np.complex128,
    )


def _gate_consts(thetas, phis, lams):
    """Constant moving-operand matrices for both PE stages + bdones."""
    thetas = np.asarray(thetas, dtype=np.float64)
    phis = np.asarray(phis, dtype=np.float64)
    lams = np.asarray(lams, dtype=np.float64)
    gates = []
    for q in range(NUM_QUBITS):
        g1 = _u3(thetas[0, q], phis[0, q], lams[0, q])
        g2 = _u3(thetas[1, q], phis[1, q], lams[1, q])
        gates.append(g2 @ g1)  # layer 1 applied first, then layer 2

    m5a = gates[0]
    for q in range(1, 5):
        m5a = np.kron(m5a, gates[q])  # [32,32], acts on q5 (bits 0-4)
    m7b = gates[5]
    for q in range(6, 12):
        m7b = np.kron(m7b, gates[q])  # [128,128], acts on l7 (bits 5-11)

    g5 = np.kron(np.eye(4), m5a)  # [128,128] block-diag over (b2, q5)

    mv1 = np.concatenate([g5.T.real, g5.T.imag], axis=1)  # [128,256]
    mv2a = np.concatenate([m7b.T.real, m7b.T.imag], axis=1)
    mv2b = np.concatenate([-m7b.T.imag, m7b.T.real], axis=1)
    bdones = np.kron(np.eye(4), np.ones((32, 32)))  # sums over q5 per b2
    bf = ml_dtypes.bfloat16
    return (
        np.ascontiguousarray(mv1, dtype=np.float32),
        np.ascontiguousarray(mv2a, dtype=np.float32).astype(bf),
        np.ascontiguousarray(mv2b, dtype=np.float32).astype(bf),
        np.ascontiguousarray(bdones, dtype=np.float32).astype(bf),
    )


def _build_nc():
    nc = bacc.Bacc(
        "TRN2", target_bir_lowering=False, debug=False, num_devices=N_CORES
    )
    x_ap = nc.dram_tensor("x", [B_CORE, D], F32R, kind="ExternalInput").ap()
    mv1_ap = nc.dram_tensor("mv1", [128, 256], F32R, kind="ExternalInput").ap()
    mv2a_ap = nc.dram_tensor("mv2a", [128, 256], BF16, kind="ExternalInput").ap()
    mv2b_ap = nc.dram_tensor("mv2b", [128, 256], BF16, kind="ExternalInput").ap()
    bd_ap = nc.dram_tensor("bdones", [128, 128], BF16, kind="ExternalInput").ap()
    out_ap = nc.dram_tensor("probs", [B_CORE, D], F32, kind="ExternalOutput").ap()

    with tile.TileContext(nc) as tc, ExitStack() as ctx:
        consts = ctx.enter_context(tc.tile_pool(name="consts", bufs=1))
        xpool = ctx.enter_context(tc.tile_pool(name="xp", bufs=2))
        s1pool = ctx.enter_context(tc.tile_pool(name="s1p", bufs=2))
        x2pool = ctx.enter_context(tc.tile_pool(name="x2p", bufs=2))
        segpool = ctx.enter_context(tc.tile_pool(name="segp", bufs=2))
        invpool = ctx.enter_context(tc.tile_pool(name="invp", bufs=2))
        tpool = ctx.enter_context(tc.tile_pool(name="tp", bufs=4))
        pfpool = ctx.enter_context(tc.tile_pool(name="pfp", bufs=2))
        pfspool = ctx.enter_context(tc.tile_pool(name="pfsp", bufs=2))
        ps1 = ctx.enter_context(tc.tile_pool(name="ps1", bufs=2, space="PSUM"))
        ps2 = ctx.enter_context(tc.tile_pool(name="ps2", bufs=2, space="PSUM"))

        # ---- warmup: keep PE busy (HAM un-throttle) + ACT table preload ----
        wsrc = consts.tile([128, 256], F32R, tag="wsrc")
        nc.vector.memset(wsrc[:].bitcast(F32), 0.0)
        wact = consts.tile([128, 16], BF16, tag="wact")
        nc.scalar.square(wact[:], wsrc[:, 0:16].bitcast(F32))
        pw = ps2.tile([128, 1024], F32, tag="g2")
        for i in range(24):
            nc.tensor.matmul(
                pw[:, (i % 4) * 256 : (i % 4 + 1) * 256],
                lhsT=wsrc[:, 0:128],
                rhs=wsrc[:],
                start=True,
                stop=True,
            )

        # ---- constants ----
        mv1_tt = consts.tile([128, 256], F32R, tag="mv1")
        nc.scalar.dma_start(mv1_tt[:], mv1_ap[:])
        mv2a_tt = consts.tile([128, 256], BF16, tag="mv2a")
        nc.scalar.dma_start(mv2a_tt[:], mv2a_ap[:])
        mv2b_tt = consts.tile([128, 256], BF16, tag="mv2b")
        nc.scalar.dma_start(mv2b_tt[:], mv2b_ap[:])
        bd_tt = consts.tile([128, 128], BF16, tag="bd")
        nc.scalar.dma_start(bd_tt[:], bd_ap[:])
        mv1_t = mv1_tt[:]
        mv2a_t = mv2a_tt[:]
        mv2b_t = mv2b_tt[:]

        all_X = [None] * N_CHUNKS
        all_S1 = [None] * N_CHUNKS
        all_seg = [None] * N_CHUNKS
        all_inv = [None] * N_CHUNKS
        all_Pf = [[None, None] for _ in range(N_CHUNKS)]

        def emit_load(k, pieces=2, eng=None):
            X = xpool.tile([128, D], F32R, tag="X")
            all_X[k] = X
            src = (
                x_ap[k * CHUNK : (k + 1) * CHUNK, :]
                .flatten()
                .rearrange("(bh b2 q5 l) -> (b2 q5) bh l", bh=32, b2=4, q5=32, l=128)
            )
            w = 32 // pieces
            dma_eng = eng if eng is not None else nc.sync
            for h in range(pieces):
                dma_eng.dma_start(
                    X[:, h * w * 128 : (h + 1) * w * 128].rearrange(
                        "p (bh l) -> p bh l", l=128
                    ),
                    src[:, h * w : (h + 1) * w, :],
                )

        def s1_group(k, g):
            """Stage-1 matmuls for 4 bh + psum->S1 bf16 evacuation."""
            if g == 0:
                S1 = s1pool.tile([128, 8192], BF16, tag="S1")
                all_S1[k] = S1
            X = all_X[k]
            S1 = all_S1[k]
            pg = ps1.tile([128, 1024], F32, tag="g1")
            for j in range(4):
                nc.tensor.matmul(
                    pg[:, j * 256 : (j + 1) * 256],
                    lhsT=X[:, (4 * g + j) * 128 : (4 * g + j + 1) * 128],
                    rhs=mv1_t,
                    start=True,
                    stop=True,
                )
            s1c = S1[:, g * 1024 : (g + 1) * 1024]
            if g in EVAC_V:
                nc.vector.tensor_copy(s1c, pg[:])
            else:
                nc.scalar.copy(s1c, pg[:])

        def norm_front(k, q):
            """x^2 (S) + per-bh reduce over l7 (V) for quarter q."""
            if q == 0:
                seg = segpool.tile([128, 32], BF16, tag="seg")
                all_seg[k] = seg
            X = all_X[k]
            seg = all_seg[k]
            x2 = x2pool.tile([128, 1024], BF16, tag="x2")
            nc.scalar.square(x2[:], X[:, q * 1024 : (q + 1) * 1024].bitcast(F32))
            with nc.allow_low_precision(reason="norm partials, 2e-2 tol"):
                nc.vector.tensor_reduce(
                    seg[:, q * 8 : (q + 1) * 8],
                    x2[:].rearrange("p (bh l) -> p bh l", l=128),
                    axis=mybir.AxisListType.X,
                    op=AluOpType.add,
                )

        def norm_tail(k):
            """Sum over q5 (PE block-diag ones) + reciprocal -> 1/||x||^2."""
            invn2 = invpool.tile([128, 32], F32, tag="invn2")
            all_inv[k] = invn2
            psv = ps1.tile([128, 1024], F32, tag="g1")
            nc.tensor.matmul(
                psv[:, 0:32], lhsT=bd_tt[:], rhs=all_seg[k][:], start=True, stop=True
            )
            nc.vector.reciprocal(invn2[:], psv[:, 0:32])

        def s2_group(k, g):
            """Stage-2 matmuls + squares + pair-add for 4 bh."""
            S1 = all_S1[k]
            if g % 4 == 0:
                pf_t = pfpool.tile([128, 2048], BF16, tag="Pf")
                all_Pf[k][g // 4] = pf_t
            pg2 = ps2.tile([128, 1024], F32, tag="g2")
            for j in range(4):
                base = g * 1024 + j * 256
                nc.tensor.matmul(
                    pg2[:, j * 256 : (j + 1) * 256],
                    lhsT=S1[:, base : base + 128],
                    rhs=mv2a_t,
                    start=True,
                    stop=False,
                )
                nc.tensor.matmul(
                    pg2[:, j * 256 : (j + 1) * 256],
                    lhsT=S1[:, base + 128 : base + 256],
                    rhs=mv2b_t,
                    start=False,
                    stop=True,
                )
            T = tpool.tile([128, 1024], BF16, tag="T")
            nc.scalar.square(T[:], pg2[:])
            T4 = T[:].rearrange("p (j r c) -> p j r c", j=4, r=2)
            # last chunk: V is idle during drain, G would straggle
            if k == N_CHUNKS - 1:
                add_eng = nc.vector if g % 2 else nc.gpsimd
            else:
                add_eng = nc.vector if g in ADD_V else nc.gpsimd
            add_eng.tensor_tensor(
                all_Pf[k][g // 4][:, (g % 4) * 512 : (g % 4 + 1) * 512].rearrange(
                    "p (j c) -> p j c", c=128
                ),
                T4[:, :, 0],
                T4[:, :, 1],
                op=AluOpType.add,
            )

        def scale_store(k, h):
            """1/||x||^2 scale (GpSimd; split with V on the drain chunk),
            then DMA the half back out."""
            pfs_t = pfspool.tile([128, 2048], F32, tag="PfS")
            eng = nc.vector if (k == N_CHUNKS - 1 and h == 1) else nc.gpsimd
            eng.tensor_tensor(
                pfs_t[:].rearrange("p (bh l) -> p bh l", l=128),
                all_Pf[k][h][:].rearrange("p (bh l) -> p bh l", l=128),
                all_inv[k][:, h * 16 : (h + 1) * 16]
                .unsqueeze(2)
                .broadcast_to([128, 16, 128]),
                op=AluOpType.mult,
            )
            oflat = (
                out_ap[k * CHUNK : (k + 1) * CHUNK, :]
                .flatten()
                .rearrange("(bh b2 q5 l) -> (b2 q5) bh l", bh=32, b2=4, q5=32, l=128)
            )
            nc.sync.dma_start(
                oflat[:, h * 16 : (h + 1) * 16, :],
                pfs_t[:].rearrange("p (bh l) -> p bh l", l=128),
            )

        # ---- flat software pipeline over (chunk, group) steps ----
        # s1(t) runs OFF steps ahead of s2(t-OFF); every cross-engine dep
        # gets ~OFF group-times of slack without doubling fill/drain.
        OFF = 6
        emit_load(0, pieces=4, eng=nc.scalar)
        emit_load(1, eng=nc.scalar)
        for t in range(N_CHUNKS * NG + OFF):
            if t >= OFF:
                k2, g2 = divmod(t - OFF, NG)
                s2_group(k2, g2)
            if t < N_CHUNKS * NG:
                k1, g1 = divmod(t, NG)
                s1_group(k1, g1)
                if g1 == 2:
                    norm_front(k1, 0)
                    norm_front(k1, 1)
                elif g1 == 4:
                    norm_front(k1, 2)
                    norm_front(k1, 3)
                elif g1 == 6:
                    norm_tail(k1)
                elif g1 == 7 and k1 + 2 < N_CHUNKS:
                    emit_load(k1 + 2)
            if t >= OFF:
                if g2 == 3:
                    scale_store(k2, 0)
                elif g2 == 7:
                    scale_store(k2, 1)

    nc.compile()
    return nc


_NC_CACHE = {}


def _get_nc():
    if "nc" not in _NC_CACHE:
        _NC_CACHE["nc"] = _build_nc()
    return _NC_CACHE["nc"]


def kernel(inputs, thetas, phis, lams, _trace=False, _trace_kwargs=None):
    inputs = np.ascontiguousarray(np.asarray(inputs), dtype=np.float32)
    mv1, mv2a, mv2b, bdones = _gate_consts(thetas, phis, lams)

    nc = _get_nc()
    in_maps = [
        {
            "x": inputs[k * B_CORE : (k + 1) * B_CORE],
            "mv1": mv1,
            "mv2a": mv2a,
            "mv2b": mv2b,
            "bdones": bdones,
        }
        for k in range(N_CORES)
    ]
    res = run_bass_kernel_spmd(
        nc, in_maps, list(range(N_CORES)), trace=_trace, **(_trace_kwargs or {})
    )
    out = np.concatenate([res.results[k]["probs"] for k in range(N_CORES)], axis=0)
    if _trace:
        kernel.last_result = res
    return out


# revision 15
# speedup vs baseline: 1.1828x; 1.1828x over previous
"""Trainium2 Bass kernel for the DifferentiableQuantumCircuit problem.

Math: output = |U x / ||x|| |^2 with U = kron of 12 single-qubit U3 gates
applied twice (2 layers). Gates on different qubits commute, so the two
layers fuse into ONE kron-product unitary with per-qubit gates
G_q = U3_layer2(q) @ U3_layer1(q).

State index split: i = q5 * 128 + l7, with q5 = qubits 0-4 (5 MSBs) and
l7 = qubits 5-11 (7 LSBs, contiguous in memory -> 512B DMA bursts).
U_total = M5a (x) M7b with M5a = kron(G_0..G_4) [32x32] acting on q5 and
M7b = kron(G_5..G_11) [128x128] acting on l7.

Per-core dataflow (512 samples/core, 4 chunks of 128 samples b=(bh,b2),
bh in [0,32), b2 in [0,4)); per chunk, 8 groups of 4 bh:
  stage 1 (PE, f32r): stationary = X c-tile (fixed bh), moving =
    [Re(G5^T)|Im(G5^T)] with G5 = I4 (x) M5a -> psum[l7, (re/im,(b2,q5))]
    (applies the 5-qubit gate group AND transposes l7 onto partitions)
  evac (V/S): psum f32 -> SBUF bf16 S1 tiles
  stage 2 (PE, bf16): stationary = S1 re/im slices, moving =
    [Re(M7b^T)|Im(M7b^T)] / [-Im|Re] accumulating -> psum[(b2,q5'), (re/im, l7')]
  squares (S): psum f32 -> T bf16; pair add (V/G) -> Pf bf16
  norm (off critical path): x^2 (S) -> per-bh l7-reduce (V) ->
    block-diag-ones matmul (PE) -> reciprocal (V) = 1/||x||^2
  final scale (G): Pf * invnorm2 broadcast -> PfS f32 -> DMA store

SOFTWARE PIPELINE: engine queues are FIFO in emission order, so the
main loop k interleaves stage-2 of chunk k with stage-1 of chunk k+1.
Every cross-engine dependency then has ~a full chunk of slack instead
of serializing a per-group latency chain. Norm chain for chunk k+1 is
also emitted inside loop k, placed where its inputs have landed.
"""

from contextlib import ExitStack

import numpy as np
import ml_dtypes

import concourse.tile as tile
from concourse import bacc, mybir
from concourse.alu_op_type import AluOpType
from concourse.bass_utils import run_bass_kernel_spmd

F32 = mybir.dt.float32
F32R = mybir.dt.float32r
BF16 = mybir.dt.bfloat16

NUM_QUBITS = 12
D = 4096
B = 4096
N_CORES = 8
B_CORE = B // N_CORES  # 512
CHUNK = 128
N_CHUNKS = B_CORE // CHUNK  # 4
NG = 8  # groups per chunk (4 bh each)

EVAC_V = (0, 1, 2, 4, 5, 6, 7)  # stage-1 evacuation on VectorE; rest ScalarE
ADD_V = (0, 1, 2)  # pair-adds on VectorE (early groups fill V's loop start)


def _u3(theta, phi, lam):
    c = np.cos(theta / 2.0)
    s = np.sin(theta / 2.0)
    return np.array(
        [
            [c, -np.exp(1j * lam) * s],
            [np.exp(1j * phi) * s, np.exp(1j * (phi + lam)) * c],
        ],
        dtype=np.complex128,
    )


def _gate_consts(thetas, phis, lams):
    """Constant moving-operand matrices for both PE stages + bdones."""
    thetas = np.asarray(thetas, dtype=np.float64)
    phis = np.asarray(phis, dtype=np.float64)
    lams = np.asarray(lams, dtype=np.float64)
    gates = []
    for q in range(NUM_QUBITS):
        g1 = _u3(thetas[0, q], phis[0, q], lams[0, q])
        g2 = _u3(thetas[1, q], phis[1, q], lams[1, q])
        gates.append(g2 @ g1)  # layer 1 applied first, then layer 2

    m5a = gates[0]
    for q in range(1, 5):
        m5a = np.kron(m5a, gates[q])  # [32,32], acts on q5 (bits 0-4)
    m7b = gates[5]
    for q in range(6, 12):
        m7b = np.kron(m7b, gates[q])  # [128,128], acts on l7 (bits 5-11)

    g5 = np.kron(np.eye(4), m5a)  # [128,128] block-diag over (b2, q5)

    mv1 = np.concatenate([g5.T.real, g5.T.imag], axis=1)  # [128,256]
    mv2a = np.concatenate([m7b.T.real, m7b.T.imag], axis=1)
    mv2b = np.concatenate([-m7b.T.imag, m7b.T.real], axis=1)
    bdones = np.kron(np.eye(4), np.ones((32, 32)))  # sums over q5 per b2
    bf = ml_dtypes.bfloat16
    return (
        np.ascontiguousarray(mv1, dtype=np.float32),
        np.ascontiguousarray(mv2a, dtype=np.float32).astype(bf),
        np.ascontiguousarray(mv2b, dtype=np.float32).astype(bf),
        np.ascontiguousarray(bdones, dtype=np.float32).astype(bf),
    )


def _build_nc():
    nc = bacc.Bacc(
        "TRN2", target_bir_lowering=False, debug=False, num_devices=N_CORES
    )
    x_ap = nc.dram_tensor("x", [B_CORE, D], F32R, kind="ExternalInput").ap()
    mv1_ap = nc.dram_tensor("mv1", [128, 256], F32R, kind="ExternalInput").ap()
    mv2a_ap = nc.dram_tensor("mv2a", [128, 256], BF16, kind="ExternalInput").ap()
    mv2b_ap = nc.dram_tensor("mv2b", [128, 256], BF16, kind="ExternalInput").ap()
    bd_ap = nc.dram_tensor("bdones", [128, 128], BF16, kind="ExternalInput").ap()
    out_ap = nc.dram_tensor("probs", [B_CORE, D], F32, kind="ExternalOutput").ap()

    with tile.TileContext(nc) as tc, ExitStack() as ctx:
        consts = ctx.enter_context(tc.tile_pool(name="consts", bufs=1))
        xpool = ctx.enter_context(tc.tile_pool(name="xp", bufs=2))
        s1pool = ctx.enter_context(tc.tile_pool(name="s1p", bufs=2))
        x2pool = ctx.enter_context(tc.tile_pool(name="x2p", bufs=2))
        segpool = ctx.enter_context(tc.tile_pool(name="segp", bufs=2))
        invpool = ctx.enter_context(tc.tile_pool(name="invp", bufs=2))
        tpool = ctx.enter_context(tc.tile_pool(name="tp", bufs=4))
        pfpool = ctx.enter_context(tc.tile_pool(name="pfp", bufs=2))
        pfspool = ctx.enter_context(tc.tile_pool(name="pfsp", bufs=2))
        ps1 = ctx.enter_context(tc.tile_pool(name="ps1", bufs=2, space="PSUM"))
        ps2 = ctx.enter_context(tc.tile_pool(name="ps2", bufs=2, space="PSUM"))

        # ---- warmup: keep PE busy (HAM un-throttle) + ACT table preload ----
        wsrc = consts.tile([128, 256], F32R, tag="wsrc")
        nc.vector.memset(wsrc[:].bitcast(F32), 0.0)
        wact = consts.tile([128, 16], BF16, tag="wact")
        nc.scalar.square(wact[:], wsrc[:, 0:16].bitcast(F32))
        pw = ps2.tile([128, 1024], F32, tag="g2")
        for i in range(24):
            nc.tensor.matmul(
                pw[:, (i % 4) * 256 : (i % 4 + 1) * 256],
                lhsT=wsrc[:, 0:128],
                rhs=wsrc[:],
                start=True,
                stop=True,
            )

        # ---- constants ----
        mv1_tt = consts.tile([128, 256], F32R, tag="mv1")
        nc.sync.dma_start(mv1_tt[:], mv1_ap[:])
        mv2a_tt = consts.tile([128, 256], BF16, tag="mv2a")
        nc.sync.dma_start(mv2a_tt[:], mv2a_ap[:])
        mv2b_tt = consts.tile([128, 256], BF16, tag="mv2b")
        nc.sync.dma_start(mv2b_tt[:], mv2b_ap[:])
        bd_tt = consts.tile([128, 128], BF16, tag="bd")
        nc.sync.dma_start(bd_tt[:], bd_ap[:])
        mv1_t = mv1_tt[:]
        mv2a_t = mv2a_tt[:]
        mv2b_t = mv2b_tt[:]

        all_X = [None] * N_CHUNKS
        all_S1 = [None] * N_CHUNKS
        all_seg = [None] * N_CHUNKS
        all_x2 = [None] * N_CHUNKS
        all_inv = [None] * N_CHUNKS
        all_Pf = [[None, None] for _ in range(N_CHUNKS)]

        def emit_load(k, pieces=2, eng=None):
            X = xpool.tile([128, D], F32R, tag="X")
            all_X[k] = X
            src = (
                x_ap[k * CHUNK : (k + 1) * CHUNK, :]
                .flatten()
                .rearrange("(bh b2 q5 l) -> (b2 q5) bh l", bh=32, b2=4, q5=32, l=128)
            )
            w = 32 // pieces
            dma_eng = eng if eng is not None else nc.sync
            for h in range(pieces):
                dma_eng.dma_start(
                    X[:, h * w * 128 : (h + 1) * w * 128].rearrange(
                        "p (bh l) -> p bh l", l=128
                    ),
                    src[:, h * w : (h + 1) * w, :],
                )

        def s1_group(k, g):
            """Stage-1 matmuls for 4 bh + psum->S1 bf16 evacuation."""
            if g == 0:
                S1 = s1pool.tile([128, 8192], BF16, tag="S1")
                all_S1[k] = S1
            X = all_X[k]
            S1 = all_S1[k]
            pg = ps1.tile([128, 1024], F32, tag="g1")
            for j in range(4):
                nc.tensor.matmul(
                    pg[:, j * 256 : (j + 1) * 256],
                    lhsT=X[:, (4 * g + j) * 128 : (4 * g + j + 1) * 128],
                    rhs=mv1_t,
                    start=True,
                    stop=True,
                )
            s1c = S1[:, g * 1024 : (g + 1) * 1024]
            if g in EVAC_V:
                nc.vector.tensor_copy(s1c, pg[:])
            else:
                nc.scalar.copy(s1c, pg[:])

        def norm_front(k, q):
            """x^2 on ScalarE for quarter q into the per-chunk x2 tile."""
            if q == 0:
                x2 = x2pool.tile([128, 4096], BF16, tag="x2")
                all_x2[k] = x2
            X = all_X[k]
            nc.scalar.square(
                all_x2[k][:, q * 1024 : (q + 1) * 1024],
                X[:, q * 1024 : (q + 1) * 1024].bitcast(F32),
            )

        def norm_tail(k):
            """One l7-reduce (V), sum over q5 (PE), reciprocal -> 1/||x||^2."""
            seg = segpool.tile([128, 32], BF16, tag="seg")
            all_seg[k] = seg
            with nc.allow_low_precision(reason="norm partials, 2e-2 tol"):
                nc.vector.tensor_reduce(
                    seg[:],
                    all_x2[k][:].rearrange("p (bh l) -> p bh l", l=128),
                    axis=mybir.AxisListType.X,
                    op=AluOpType.add,
                )
            invn2 = invpool.tile([128, 32], F32, tag="invn2")
            all_inv[k] = invn2
            psv = ps1.tile([128, 1024], F32, tag="g1")
            nc.tensor.matmul(
                psv[:, 0:32], lhsT=bd_tt[:], rhs=seg[:], start=True, stop=True
            )
            nc.vector.reciprocal(invn2[:], psv[:, 0:32])

        def s2_group(k, g):
            """Stage-2 matmuls + squares + pair-add for 4 bh."""
            S1 = all_S1[k]
            if g % 4 == 0:
                pf_t = pfpool.tile([128, 2048], BF16, tag="Pf")
                all_Pf[k][g // 4] = pf_t
            pg2 = ps2.tile([128, 1024], F32, tag="g2")
            for j in range(4):
                base = g * 1024 + j * 256
                nc.tensor.matmul(
                    pg2[:, j * 256 : (j + 1) * 256],
                    lhsT=S1[:, base : base + 128],
                    rhs=mv2a_t,
                    start=True,
                    stop=False,
                )
                nc.tensor.matmul(
                    pg2[:, j * 256 : (j + 1) * 256],
                    lhsT=S1[:, base + 128 : base + 256],
                    rhs=mv2b_t,
                    start=False,
                    stop=True,
                )
            T = tpool.tile([128, 1024], BF16, tag="T")
            nc.scalar.square(T[:], pg2[:])
            T4 = T[:].rearrange("p (j r c) -> p j r c", j=4, r=2)
            # last chunk: V is idle during drain, G would straggle
            if k == N_CHUNKS - 1:
                add_eng = nc.vector if g % 2 else nc.gpsimd
            else:
                add_eng = nc.vector if g in ADD_V else nc.gpsimd
            add_eng.tensor_tensor(
                all_Pf[k][g // 4][:, (g % 4) * 512 : (g % 4 + 1) * 512].rearrange(
                    "p (j c) -> p j c", c=128
                ),
                T4[:, :, 0],
                T4[:, :, 1],
                op=AluOpType.add,
            )

        def scale_store(k, h):
            """1/||x||^2 scale (GpSimd; split with V on the drain chunk),
            then DMA the half back out."""
            pfs_t = pfspool.tile([128, 2048], F32, tag="PfS")
            eng = nc.vector if (k == N_CHUNKS - 1 and h == 1) else nc.gpsimd
            eng.tensor_tensor(
                pfs_t[:].rearrange("p (bh l) -> p bh l", l=128),
                all_Pf[k][h][:].rearrange("p (bh l) -> p bh l", l=128),
                all_inv[k][:, h * 16 : (h + 1) * 16]
                .unsqueeze(2)
                .broadcast_to([128, 16, 128]),
                op=AluOpType.mult,
            )
            oflat = (
                out_ap[k * CHUNK : (k + 1) * CHUNK, :]
                .flatten()
                .rearrange("(bh b2 q5 l) -> (b2 q5) bh l", bh=32, b2=4, q5=32, l=128)
            )
            nc.sync.dma_start(
                oflat[:, h * 16 : (h + 1) * 16, :],
                pfs_t[:].rearrange("p (bh l) -> p bh l", l=128),
            )

        # ---- flat software pipeline over (chunk, group) steps ----
        # s1(t) runs OFF steps ahead of s2(t-OFF); every cross-engine dep
        # gets ~OFF group-times of slack without doubling fill/drain.
        OFF = 6
        emit_load(0, pieces=4)
        emit_load(1)
        for t in range(N_CHUNKS * NG + OFF):
            if t >= OFF:
                k2, g2 = divmod(t - OFF, NG)
                s2_group(k2, g2)
            if t < N_CHUNKS * NG:
                k1, g1 = divmod(t, NG)
                s1_group(k1, g1)
                if g1 == 2:
                    norm_front(k1, 0)
                    norm_front(k1, 1)
                elif g1 == 4:
                    norm_front(k1, 2)
                    norm_front(k1, 3)
                elif g1 == 6:
                    norm_tail(k1)
                elif g1 == 7 and k1 + 2 < N_CHUNKS:
                    emit_load(k1 + 2)
            if t >= OFF:
                if g2 == 3:
                    scale_store(k2, 0)
                elif g2 == 7:
                    scale_store(k2, 1)

    nc.compile()
    return nc


_NC_CACHE = {}


def _get_nc():
    if "nc" not in _NC_CACHE:
        _NC_CACHE["nc"] = _build_nc()
    return _NC_CACHE["nc"]


def kernel(inputs, thetas, phis, lams, _trace=False, _trace_kwargs=None):
    inputs = np.ascontiguousarray(np.asarray(inputs), dtype=np.float32)
    mv1, mv2a, mv2b, bdones = _gate_consts(thetas, phis, lams)

    nc = _get_nc()
    in_maps = [
        {
            "x": inputs[k * B_CORE : (k + 1) * B_CORE],
            "mv1": mv1,
            "mv2a": mv2a,
            "mv2b": mv2b,
            "bdones": bdones,
        }
        for k in range(N_CORES)
    ]
    res = run_bass_kernel_spmd(
        nc, in_maps, list(range(N_CORES)), trace=_trace, **(_trace_kwargs or {})
    )
    out = np.concatenate([res.results[k]["probs"] for k in range(N_CORES)], axis=0)
    if _trace:
        kernel.last_result = res
    return out
